# revision 10
# baseline (speedup 1.0000x reference)
"""ArcFace multi-head-sharded loss on 8 TRN2 NeuronCores.

Strategy: shard the (64, 2048, 256) weight table over the group axis —
each core owns 8 groups. Samples are routed host-side to the core owning
their group (host routing replaces the all-to-all). Weight rows are
l2-normalized host-side and quantized to fp8e4 (x16 pre-scale to stay in
the normal range), so the device only does:

  - stream its 8 weight groups (4MB fp8) from HBM; DMA triggers alternate
    between the two HW-DGE queues (sync + scalar) so descriptor
    generation is not serialized on one engine,
  - mains: cos_raw(b, c) = <xq_b, wq_c> on PE (fp8 x fp8 -> f32 PSUM),
  - exp with fused accumulation over the class axis (scale folds the
    1/256 quantization scale and the ArcFace scale 64),
  - target logit via a per-row dot with the host-gathered target weight
    row (xw . wtar, 256-wide DVE reduce),
  - the margin + CE epilogue on [128,T] vectors (both tiles batched),
  - one partial-loss scalar out (sum of -logp/B over its samples).

Host: sums the 8 scalars. ~4MB HBM traffic per core => memory-bound.

Samples are packed into "bands" of NG=32 partition rows, one band per
weight group (plus overflow bands), 4 bands per 128-row sample tile.
"""

import sys
import numpy as np
import ml_dtypes

BF16 = ml_dtypes.bfloat16
FP8 = ml_dtypes.float8_e4m3

_TRN_REPO = "/opt/trn_rl_repo"
if _TRN_REPO not in sys.path:
    sys.path.insert(0, _TRN_REPO)

# problem config (hardcoded per spec)
B, E, G, C = 512, 256, 64, 2048
NCORES = 8
GPC = G // NCORES        # weight groups per core
NG = 32                  # sample slots per band
BPT = 128 // NG          # bands per 128-partition sample tile
KE = E // 128            # contraction chunks
NCC = C // 512           # 512-col chunks per group
SCALE = 64.0
MARGIN = 0.5
COS_M = float(np.cos(MARGIN))
SIN_M = float(np.sin(MARGIN))
THETA = float(np.cos(np.pi - MARGIN))
SINMM = float(np.sin(np.pi - MARGIN) * MARGIN)
EPS = 1e-12
WS = 16.0                # fp8 pre-scale (per operand); PSUM = WS^2 * cos
NAUX = 2 * E + 1         # per-tile aux row: xw | wtar | redw
DOUBLE_ROW = False       # fp8 DoubleRow perf mode for the mains

_graph_cache = {}


def _build(nb, double_row=DOUBLE_ROW):
    """Build the per-core Bass graph for nb weight bands (nb % BPT == 0)."""
    from contextlib import ExitStack
    import concourse.bacc as bacc
    import concourse.tile as tile
    from concourse import mybir

    f32 = mybir.dt.float32
    bf16 = mybir.dt.bfloat16
    fp8 = mybir.dt.float8e4
    i32 = mybir.dt.int32
    A = mybir.AluOpType
    AF = mybir.ActivationFunctionType

    T = nb // BPT
    nc = bacc.Bacc(None)

    wt_ext = nc.declare_dram_parameter("wt", [nb, 128, KE, C], fp8, isOutput=False)
    xt_ext = nc.declare_dram_parameter("xt", [128, T, KE, 128], fp8, isOutput=False)
    aux_ext = nc.declare_dram_parameter("aux", [128, T, NAUX], bf16, isOutput=False)
    out_ext = nc.declare_dram_parameter("out", [1, 1], f32, isOutput=True)

    with tile.TileContext(nc) as tc, ExitStack() as ctx:
        wpool = ctx.enter_context(tc.tile_pool(name="w", bufs=nb))
        cpool = ctx.enter_context(tc.tile_pool(name="const", bufs=1))
        vpool = ctx.enter_context(tc.tile_pool(name="vec", bufs=2))
        pmain = ctx.enter_context(tc.tile_pool(name="pmain", bufs=7, space="PSUM"))
        ploss = ctx.enter_context(tc.tile_pool(name="ploss", bufs=1, space="PSUM"))

        # DMA triggers: bands alternate sync/scalar HW-DGE rings so
        # descriptor generation is parallel and bands arrive in index order;
        # xt first on scalar (mains need it), aux early for the margin
        # pre-compute.
        w_tiles = []
        for b in range(nb):
            wt = wpool.tile([128, KE, C], fp8, tag="wt", name=f"wt{b}")
            w_tiles.append(wt)
        xt_sb = cpool.tile([128, T, KE, 128], fp8, tag="xt")
        aux_sb = cpool.tile([128, T, NAUX], bf16, tag="aux")

        nc.sync.dma_start(out=w_tiles[0][:], in_=wt_ext[0])
        nc.scalar.dma_start(out=xt_sb[:], in_=xt_ext[:])
        nc.scalar.dma_start(out=w_tiles[1][:], in_=wt_ext[1])
        nc.sync.dma_start(out=w_tiles[2][:], in_=wt_ext[2])
        nc.scalar.dma_start(out=w_tiles[3][:], in_=wt_ext[3])
        nc.sync.dma_start(out=aux_sb[:], in_=aux_ext[:])
        for b in range(5, nb, 2):
            nc.scalar.dma_start(out=w_tiles[b][:], in_=wt_ext[b])
        for b in range(4, nb, 2):
            nc.sync.dma_start(out=w_tiles[b][:], in_=wt_ext[b])

        # preload the natural_log_exp_and_others ACT table set (exp, ln):
        # one resident set => zero mid-kernel table loads. After the scalar
        # queue's DMA triggers so they are not delayed.
        nc.scalar.add_instruction(mybir.InstLoadActFuncSet(
            name="preload-actset-6", act_func_set_id=6, ins=[], outs=[]))

        # margin pre-compute, both tiles batched as [128, T] columns:
        # t = <xn, wn_target>; ft = t>theta ? t*cos_m - sqrt(1-t^2)*sin_m
        #                                  : t - sinmm   (labels always valid)
        tcos = cpool.tile([128, T], f32, tag="tcos")
        for t in range(T):
            tscr = vpool.tile([128, E], f32, tag="tscr")
            nc.vector.tensor_tensor(tscr[:], aux_sb[:, t, 0:E],
                                    aux_sb[:, t, E:2 * E], A.mult)
            nc.vector.reduce_sum(tcos[:, t:t + 1], tscr[:], axis=mybir.AxisListType.X)
        t2 = vpool.tile([128, T], f32, tag="t2")
        nc.vector.tensor_tensor(t2[:], tcos[:], tcos[:], A.mult)
        nc.vector.tensor_scalar(t2[:], t2[:], -1.0, 1.0, op0=A.mult, op1=A.add)
        nc.vector.tensor_scalar_max(t2[:], t2[:], 0.0)
        # sin_t = z*rsqrt(z): Quake seed + 2 Newton iterations on DVE
        yrs = vpool.tile([128, T], f32, tag="yrs")
        yi = yrs.bitcast(i32)
        nc.vector.tensor_scalar(yi[:], t2.bitcast(i32)[:], 1, None, op0=A.arith_shift_right)
        nc.vector.tensor_scalar(yi[:], yi[:], -1, 0x5F3759DF, op0=A.mult, op1=A.add)
        hz = vpool.tile([128, T], f32, tag="hz")
        nc.vector.tensor_scalar_mul(hz[:], t2[:], 0.5)
        y2 = vpool.tile([128, T], f32, tag="y2")
        for _ in range(2):
            nc.vector.tensor_tensor(y2[:], yrs[:], yrs[:], A.mult)
            nc.vector.tensor_tensor(y2[:], y2[:], hz[:], A.mult)
            nc.vector.tensor_scalar(y2[:], y2[:], -1.0, 1.5, op0=A.mult, op1=A.add)
            nc.vector.tensor_tensor(yrs[:], yrs[:], y2[:], A.mult)
        sint = vpool.tile([128, T], f32, tag="sint")
        nc.vector.tensor_tensor(sint[:], t2[:], yrs[:], A.mult)
        ctm = vpool.tile([128, T], f32, tag="ctm")
        nc.vector.tensor_scalar_mul(ctm[:], tcos[:], COS_M)
        sinm = vpool.tile([128, T], f32, tag="sinm")
        nc.vector.tensor_scalar_mul(sinm[:], sint[:], SIN_M)
        nc.vector.tensor_tensor(ctm[:], ctm[:], sinm[:], A.subtract)
        tms = vpool.tile([128, T], f32, tag="tms")
        nc.vector.tensor_scalar_add(tms[:], tcos[:], -SINMM)
        gt = vpool.tile([128, T], i32, tag="gt")
        nc.vector.tensor_scalar(gt[:], tcos[:], THETA, None, op0=A.is_gt)
        ft = vpool.tile([128, T], f32, tag="ft")
        nc.vector.select(ft[:], gt[:], ctm[:], tms[:])
        tf64 = cpool.tile([128, 2 * T], f32, tag="tf64")
        nc.vector.tensor_scalar_mul(tf64[:, 0:T], tcos[:], SCALE)
        nc.vector.tensor_scalar_mul(tf64[:, T:2 * T], ft[:], SCALE)
        eb = cpool.tile([128, 2 * T], f32, tag="eb")
        nc.scalar.activation(eb[:], tf64[:], AF.Exp)
        # per tile t: exp(64t) = eb[:, t], exp(64ft) = eb[:, T+t],
        #             64ft = tf64[:, T+t]

        loss_ps = ploss.tile([1, 1], f32, tag="loss")
        escale = SCALE / (WS * WS)   # exp(escale * psum) = exp(64*cos)

        cps_t = {}

        def emit_mains(t):
            """Matmul order (cc, k, j): j innermost so the 4 bands' matmuls
            run concurrently on distinct PE column quadrants, cc outermost so
            PSUM chunks complete (and exp) one at a time."""
            cps_t[t] = [pmain.tile([128, 512], f32, tag="cos", name=f"cos{t}_{cc}")
                        for cc in range(NCC)]
            cps = cps_t[t]
            if double_row:
                for cc in range(NCC):
                    for j in range(BPT):
                        nc.tensor.matmul(
                            cps[cc][NG * j:NG * (j + 1), :],
                            xt_sb[:, t, 0:KE, NG * j: NG * (j + 1)],
                            w_tiles[BPT * t + j][:, 0:KE, 512 * cc: 512 * cc + 512],
                            start=True, stop=True,
                            perf_mode=mybir.MatmulPerfMode.DoubleRow,
                            tile_position=(0, NG * j),
                        )
            else:
                for cc in range(NCC):
                    for k in range(KE):
                        for j in range(BPT):
                            nc.tensor.matmul(
                                cps[cc][NG * j:NG * (j + 1), :],
                                xt_sb[:, t, k, NG * j: NG * (j + 1)],
                                w_tiles[BPT * t + j][:, k, 512 * cc: 512 * cc + 512],
                                start=(k == 0), stop=(k == KE - 1),
                                tile_position=(0, NG * j),
                            )

        def emit_tail(t):
            """exp/accum + CE epilogue for sample tile t (ACT + DVE; the
            loss matmul lands after all mains in the PE queue)"""
            cps = cps_t[t]
            # exp with fused class-axis accumulation, one per PSUM chunk
            ses = cpool.tile([128, NCC], f32, tag=f"ses{t}")
            for cc in range(NCC):
                escr = vpool.tile([128, 512], bf16, tag="escr")
                nc.scalar.activation(escr[:], cps[cc][:], AF.Exp, scale=escale,
                                     accum_out=ses[:, cc:cc + 1])
            sumexp = vpool.tile([128, 1], f32, tag="sumexp")
            nc.vector.reduce_sum(sumexp[:], ses[:], axis=mybir.AxisListType.X)
            # se2 = sumexp - exp(64 t) + exp(64 ft);  lb = ln(se2) - 64 ft
            se2 = vpool.tile([128, 1], f32, tag="se2")
            nc.vector.tensor_tensor(se2[:], sumexp[:], eb[:, t:t + 1], A.subtract)
            nc.vector.tensor_tensor(se2[:], se2[:], eb[:, T + t:T + t + 1], A.add)
            lse = vpool.tile([128, 1], f32, tag="lse")
            nc.scalar.activation(lse[:], se2[:], AF.Ln)
            lb = cpool.tile([128, 1], bf16, tag=f"lb{t}")
            nc.vector.tensor_tensor(lb[:], lse[:], tf64[:, T + t:T + t + 1], A.subtract)
            nc.tensor.matmul(
                loss_ps[:], aux_sb[:, t, 2 * E:NAUX], lb[:],
                start=(t == 0), stop=(t == T - 1),
            )

        for t in range(T):
            emit_mains(t)
        for t in range(T):
            emit_tail(t)

        loss_sb = cpool.tile([1, 1], f32, tag="losssb")
        nc.vector.tensor_copy(loss_sb[:], loss_ps[:])
        nc.sync.dma_start(out=out_ext[:], in_=loss_sb[:])

    nc.compile()
    return nc


def _pack(logits, labels, weight):
    """Route samples to the core owning their group; build per-core inputs."""
    logits = np.asarray(logits, dtype=np.float32)
    labels = np.asarray(labels).astype(np.int64)
    weight = np.asarray(weight, dtype=np.float32)

    group = (labels // C).astype(np.int64)
    local = (labels % C).astype(np.int64)
    core = group // GPC
    gl = group % GPC

    # host-side l2 normalization + fp8 quantization (x16 keeps the values
    # in fp8e4's normal range; cos is invariant to the row scaling)
    xn = logits / np.maximum(
        np.sqrt(np.sum(logits * logits, axis=1, keepdims=True)), EPS)
    wn2 = np.sqrt(np.einsum("gce,gce->gc", weight, weight))[:, :, None]
    wn = weight / np.maximum(wn2, EPS)
    wq = (WS * wn).astype(FP8)                    # (G, C, E) fp8 table
    xq = (WS * xn).astype(FP8)                    # (B, E)
    xw_all = (xq.astype(np.float32) / WS).astype(BF16)
    wtar_all = (wq[group, local].astype(np.float32) / WS).astype(BF16)

    # band assignment: per (core, local-group), ceil(count/NG) bands
    percg = [[np.nonzero((core == c) & (gl == g))[0] for g in range(GPC)]
             for c in range(NCORES)]
    nbands = [sum(max(1, -(-len(idx) // NG)) for idx in percg[c])
              for c in range(NCORES)]
    nb = max(nbands)
    nb = -(-nb // BPT) * BPT  # round up to full sample tiles
    T = nb // BPT

    in_maps = []
    for c in range(NCORES):
        # band -> (group, sample indices)
        bands = []
        for g in range(GPC):
            idx = percg[c][g]
            nslice = max(1, -(-len(idx) // NG))
            for s in range(nslice):
                bands.append((g, idx[s * NG:(s + 1) * NG]))
        while len(bands) < nb:
            bands.append((0, np.empty(0, dtype=np.int64)))

        wt = np.empty((nb, 128, KE, C), dtype=FP8)
        xqp = np.zeros((T, 128, E), dtype=FP8)
        aux = np.zeros((128, T, NAUX), dtype=BF16)
        for b, (g, idx) in enumerate(bands):
            wg = wq[c * GPC + g]                     # (C, E) fp8
            for k in range(KE):
                wt[b, :, k, :] = wg[:, k * 128:(k + 1) * 128].T
            t, j = b // BPT, b % BPT
            sl = slice(NG * j, NG * j + len(idx))
            xqp[t, sl, :] = xq[idx]
            aux[sl, t, 0:E] = xw_all[idx]
            aux[sl, t, E:2 * E] = wtar_all[idx]
            aux[sl, t, 2 * E] = BF16(1.0 / B)
        # xt[p, t, k, r] = xq[t][r, k*128+p]
        xt = np.ascontiguousarray(
            np.transpose(xqp.reshape(T, 128, KE, 128), (3, 0, 2, 1)))
        in_maps.append({"wt": wt, "xt": xt, "aux": aux})
    return in_maps, nb


def _run(logits, labels, weight, trace=False, **kw):
    from concourse.bass_utils import run_bass_kernel_spmd

    in_maps, nb = _pack(logits, labels, weight)
    nc = _graph_cache.get(nb)
    if nc is None:
        nc = _build(nb)
        _graph_cache[nb] = nc
    res = run_bass_kernel_spmd(nc, in_maps, core_ids=list(range(NCORES)),
                               trace=trace, **kw)
    total = sum(float(res.results[i]["out"][0, 0]) for i in range(NCORES))
    return np.asarray(total, dtype=np.float32), res


def kernel(logits, labels, weight):
    loss, _ = _run(logits, labels, weight)
    return loss
